# revision 1
# baseline (speedup 1.0000x reference)
"""TRN2 Bass kernel for nn_DifferentiablePersistentHomology_90933047591278.

kernel(**inputs) takes the FULL inputs (point_cloud [32768,1024,2] f32 plus
the tiny learned params) and returns the FULL [32768, 6] f32 output,
computed on 8 NeuronCores (pure batch data-parallel, 4096 rows per core).

Per 128-row group (one row per SBUF partition):
  scores = w0*x + w1*y  ->  exact 50th-largest threshold T via a 4-count
  Newton chain + max8/match_replace window extraction  ->  final mask ->
  prefix-scan + gpsimd local_scatter compaction of the 50 landmark indices
  ->  indirect-DMA gather of landmark coords  ->  50x50 distance stats.

Outputs: [mean, std, min, max, conn, row_std] of the scaled distance
matrix. min == distance_bias exactly (diagonal zeros) and conn == 1248/2500
identically (symmetric duplicate pairs + 50 diagonal zeros around the
lower-middle order statistic), so both are emitted as constants.
Learned-parameter scalars are baked into the compiled program as immediates.
"""
import os
import sys

if "/opt/trn_rl_repo" not in sys.path:
    sys.path.insert(0, "/opt/trn_rl_repo")

import numpy as np

N = 1024
L = 50
B_TOTAL = 32768
N_CORES = 8
NEG_BIG = -1e30
CONN_CONST = 1248.0 / 2500.0
BIAS3 = 10.0
WIN_ROUNDS = 4

TRACE = bool(int(os.environ.get("KERNEL_TRACE", "0")))
LAST = {}

_CACHE = {}


def _host_constants(attn_w, filtration_weights, distance_bias, nsim=20000):
    w0 = float(np.asarray(attn_w)[0, 0])
    w1 = float(np.asarray(attn_w)[0, 1])
    sigma = float(np.hypot(w0, w1))
    a = abs(float(np.asarray(filtration_weights)[0, 0]))
    b = float(np.asarray(distance_bias)[0])
    if sigma == 0.0:
        return dict(w0=w0, w1=w1, sigma=sigma, a=a, b=b)
    t0 = sigma * 1.268
    sim = np.random.default_rng(1).standard_normal((nsim, N)).astype(np.float32) * sigma
    c1s = (sim >= t0).sum(axis=1)
    part = np.partition(sim, (N - L - 1, N - L), axis=1)
    tgt = 0.5 * (part[:, N - L] + part[:, N - L - 1])
    co = np.polyfit(c1s.astype(np.float64), tgt, 2)
    quad = (float(co[2]), float(co[1]), float(co[0]))
    g50 = sigma / (N * 0.10226)
    return dict(w0=w0, w1=w1, sigma=sigma, a=a, b=b, t0=t0, quad=quad, g50=g50)


def _build_program(consts, n_batches):
    import concourse.bass as bass
    import concourse.bacc as bacc
    import concourse.mybir as mybir
    from concourse.bass import IndirectOffsetOnAxis
    from concourse.tile import TileContext

    F32 = mybir.dt.float32
    U32 = mybir.dt.uint32
    ALU = mybir.AluOpType
    ACTF = mybir.ActivationFunctionType

    assert n_batches % 128 == 0
    groups = n_batches // 128
    w0, w1 = consts["w0"], consts["w1"]
    a, b = consts["a"], consts["b"]
    t0 = consts["t0"]
    A0, A1, A2 = consts["quad"]
    g50 = consts["g50"]
    W = 8 * WIN_ROUNDS

    nc = bacc.Bacc()
    pc = nc.dram_tensor("pc", [n_batches, N, 2], F32, kind="ExternalInput")
    iotaneg_c = nc.dram_tensor("iotaneg_c", [1, N], F32, kind="ExternalInput")
    iota24_c = nc.dram_tensor("iota24_c", [1, W], F32, kind="ExternalInput")
    rowb_c = nc.dram_tensor("rowb_c", [128, 1], F32, kind="ExternalInput")
    out_t = nc.dram_tensor("out", [n_batches, 6], F32, kind="ExternalOutput")
    dbg_t = nc.dram_tensor("dbg", [n_batches, 2], F32, kind="ExternalOutput")

    pc_flat = pc.ap().rearrange("a b c -> (a b) c")

    with TileContext(nc) as tc:
        with tc.tile_pool(name="const", bufs=1) as cpool, \
             tc.tile_pool(name="work", bufs=2) as pool:
            iotaneg = cpool.tile([128, N], F32)
            nc.sync.dma_start(out=iotaneg[:],
                              in_=iotaneg_c.ap().broadcast_to([128, N]))
            iota24 = cpool.tile([128, W], F32)
            nc.sync.dma_start(out=iota24[:], in_=iota24_c.ap().broadcast_to([128, W]))
            rowb = cpool.tile([128, 1], F32)
            nc.sync.dma_start(out=rowb[:], in_=rowb_c[:])
            bigneg = cpool.tile([128, N], F32)
            nc.vector.memset(bigneg[:], NEG_BIG)

            for g in range(groups):
                xt = pool.tile([128, N, 2], F32)
                nc.sync.dma_start(out=xt[:], in_=pc[g * 128:(g + 1) * 128])

                ty = pool.tile([128, N], F32)
                nc.scalar.activation(out=ty[:], in_=xt[:, :, 1], func=ACTF.Copy,
                                     scale=float(w1))
                sx = pool.tile([128, N], F32)
                nc.vector.tensor_scalar(out=sx[:], in0=xt[:, :, 0],
                                        scalar1=float(w0), scalar2=None,
                                        op0=ALU.mult)
                s = pool.tile([128, N], F32)
                nc.vector.tensor_tensor(out=s[:], in0=sx[:], in1=ty[:], op=ALU.add)

                junk = pool.tile([128, N], F32)
                c1 = pool.tile([128, 1], F32)
                nc.vector.tensor_scalar(out=junk[:], in0=s[:], scalar1=float(t0),
                                        scalar2=0.0, op0=ALU.is_ge, op1=ALU.add,
                                        accum_out=c1[:])
                u1 = pool.tile([128, 1], F32)
                nc.vector.tensor_scalar(out=u1[:], in0=c1[:], scalar1=float(A2),
                                        scalar2=float(A1), op0=ALU.mult, op1=ALU.add)
                u2 = pool.tile([128, 1], F32)
                nc.vector.tensor_tensor(out=u2[:], in0=u1[:], in1=c1[:], op=ALU.mult)
                t1 = pool.tile([128, 1], F32)
                nc.vector.tensor_scalar(out=t1[:], in0=u2[:], scalar1=float(A0),
                                        scalar2=None, op0=ALU.add)
                c2 = pool.tile([128, 1], F32)
                nc.vector.tensor_scalar(out=junk[:], in0=s[:], scalar1=t1[:],
                                        scalar2=0.0, op0=ALU.is_ge, op1=ALU.add,
                                        accum_out=c2[:])
                v1 = pool.tile([128, 1], F32)
                nc.vector.tensor_scalar(out=v1[:], in0=c2[:], scalar1=-float(L),
                                        scalar2=float(g50), op0=ALU.add, op1=ALU.mult)
                t2 = pool.tile([128, 1], F32)
                nc.vector.tensor_tensor(out=t2[:], in0=v1[:], in1=t1[:], op=ALU.add)
                c3 = pool.tile([128, 1], F32)
                nc.vector.tensor_scalar(out=junk[:], in0=s[:], scalar1=t2[:],
                                        scalar2=0.0, op0=ALU.is_ge, op1=ALU.add,
                                        accum_out=c3[:])
                v2 = pool.tile([128, 1], F32)
                nc.vector.tensor_scalar(out=v2[:], in0=c3[:],
                                        scalar1=-float(L) - BIAS3,
                                        scalar2=float(g50), op0=ALU.add, op1=ALU.mult)
                t3 = pool.tile([128, 1], F32)
                nc.vector.tensor_tensor(out=t3[:], in0=v2[:], in1=t2[:], op=ALU.add)
                c4 = pool.tile([128, 1], F32)
                nc.vector.tensor_scalar(out=junk[:], in0=s[:], scalar1=t3[:],
                                        scalar2=0.0, op0=ALU.is_ge, op1=ALU.add,
                                        accum_out=c4[:])

                negs = pool.tile([128, N], F32)
                nc.vector.tensor_scalar(out=negs[:], in0=s[:], scalar1=-1.0,
                                        scalar2=None, op0=ALU.mult)
                bigm = pool.tile([128, N], F32)
                nc.vector.scalar_tensor_tensor(
                    out=bigm[:], in0=s[:], scalar=t3[:], in1=bigneg[:],
                    op0=ALU.is_lt, op1=ALU.mult)
                z0t = pool.tile([128, N], F32)
                nc.vector.tensor_tensor(out=z0t[:], in0=negs[:], in1=bigm[:],
                                        op=ALU.add)
                w24 = pool.tile([128, W], F32)
                zc = z0t
                for r in range(WIN_ROUNDS):
                    nc.vector.max(out=w24[:, 8 * r:8 * (r + 1)], in_=zc[:])
                    if r + 1 < WIN_ROUNDS:
                        zn = pool.tile([128, N], F32, tag=f"zr{r % 2}")
                        nc.vector.match_replace(
                            out=zn[:], in_to_replace=w24[:, 8 * r:8 * (r + 1)],
                            in_values=zc[:], imm_value=NEG_BIG)
                        zc = zn

                rk = pool.tile([128, 1], F32)
                nc.vector.tensor_scalar(out=rk[:], in0=c4[:], scalar1=-float(L),
                                        scalar2=None, op0=ALU.add)
                eqm = pool.tile([128, W], F32)
                nc.vector.scalar_tensor_tensor(
                    out=eqm[:], in0=iota24[:], scalar=rk[:], in1=w24[:],
                    op0=ALU.is_equal, op1=ALU.mult)
                tneg = pool.tile([128, 1], F32)
                nc.vector.tensor_reduce(out=tneg[:], in_=eqm[:],
                                        axis=mybir.AxisListType.X, op=ALU.add)

                maskf = pool.tile([128, N], F32)
                cf = pool.tile([128, 1], F32)
                nc.vector.tensor_scalar(out=maskf[:], in0=negs[:], scalar1=tneg[:],
                                        scalar2=0.0, op0=ALU.is_le, op1=ALU.add,
                                        accum_out=cf[:])
                # K = -idx - BIG*(1-mask): top-50 of K = 50 smallest selected idx
                tkm = pool.tile([128, N], F32)
                nc.vector.tensor_scalar(out=tkm[:], in0=maskf[:],
                                        scalar1=-NEG_BIG, scalar2=NEG_BIG,
                                        op0=ALU.mult, op1=ALU.add)
                kv = pool.tile([128, N], F32)
                nc.vector.tensor_tensor(out=kv[:], in0=tkm[:], in1=iotaneg[:],
                                        op=ALU.add)
                w56 = pool.tile([128, 56], F32)
                kc = kv
                for r in range(7):
                    nc.vector.max(out=w56[:, 8 * r:8 * (r + 1)], in_=kc[:])
                    if r < 6:
                        kn = pool.tile([128, N], F32, tag=f"kr{r % 2}")
                        nc.vector.match_replace(
                            out=kn[:], in_to_replace=w56[:, 8 * r:8 * (r + 1)],
                            in_values=kc[:], imm_value=NEG_BIG)
                        kc = kn

                # rowbg = rowb + g*128*N ; gidx = -w56[:, :50] + rowbg
                rowbg = pool.tile([128, 1], F32)
                nc.vector.tensor_scalar(out=rowbg[:], in0=rowb[:],
                                        scalar1=float(g * 128 * N), scalar2=None,
                                        op0=ALU.add)
                gidx = pool.tile([128, L], U32)
                nc.vector.scalar_tensor_tensor(
                    out=gidx[:], in0=w56[:, 0:L], scalar=-1.0,
                    in1=rowbg[:].broadcast_to([128, L]),
                    op0=ALU.mult, op1=ALU.add)
                lxy = pool.tile([128, L, 2], F32)
                for j in range(L):
                    nc.gpsimd.indirect_dma_start(
                        out=lxy[:, j, :], out_offset=None, in_=pc_flat,
                        in_offset=IndirectOffsetOnAxis(ap=gidx[:, j:j + 1], axis=0),
                        bounds_check=n_batches * N - 1, oob_is_err=False)

                xi = lxy[:, :, 0].unsqueeze(2).broadcast_to([128, L, L])
                xj = lxy[:, :, 0].unsqueeze(1).broadcast_to([128, L, L])
                yi = lxy[:, :, 1].unsqueeze(2).broadcast_to([128, L, L])
                yj = lxy[:, :, 1].unsqueeze(1).broadcast_to([128, L, L])
                dxt = pool.tile([128, L, L], F32, tag="pair1")
                nc.vector.tensor_tensor(out=dxt[:], in0=xi, in1=xj, op=ALU.subtract)
                dyt = pool.tile([128, L, L], F32, tag="pair2")
                nc.vector.tensor_tensor(out=dyt[:], in0=yi, in1=yj, op=ALU.subtract)
                nc.scalar.activation(out=dxt[:], in_=dxt[:], func=ACTF.Square)
                nc.scalar.activation(out=dyt[:], in_=dyt[:], func=ACTF.Square)
                d2t = pool.tile([128, L, L], F32)
                nc.vector.tensor_tensor(out=d2t[:], in0=dxt[:], in1=dyt[:],
                                        op=ALU.add)
                dist = pool.tile([128, L, L], F32, tag="pair1")
                sd = pool.tile([128, 1], F32)
                nc.scalar.activation(out=dist[:], in_=d2t[:], func=ACTF.Sqrt,
                                     accum_out=sd[:])
                maxd2 = pool.tile([128, 1], F32)
                nc.vector.tensor_reduce(out=maxd2[:], in_=d2t[:].rearrange(
                    "p a b -> p (a b)"), axis=mybir.AxisListType.X, op=ALU.max)
                rows_t = pool.tile([128, L], F32)
                nc.vector.tensor_reduce(out=rows_t[:], in_=dist[:],
                                        axis=mybir.AxisListType.X, op=ALU.add)

                negmu = pool.tile([128, 1], F32)
                nc.vector.tensor_scalar(out=negmu[:], in0=sd[:],
                                        scalar1=-1.0 / 2500.0, scalar2=None,
                                        op0=ALU.mult)
                scr = pool.tile([128, L, L], F32, tag="pair2")
                s2 = pool.tile([128, 1], F32)
                nc.scalar.activation(out=scr[:], in_=dist[:], func=ACTF.Square,
                                     bias=negmu[:], accum_out=s2[:])
                sr = pool.tile([128, 1], F32)
                nc.vector.tensor_reduce(out=sr[:], in_=rows_t[:],
                                        axis=mybir.AxisListType.X, op=ALU.add)
                negmur = pool.tile([128, 1], F32)
                nc.vector.tensor_scalar(out=negmur[:], in0=sr[:],
                                        scalar1=-1.0 / float(L), scalar2=None,
                                        op0=ALU.mult)
                scr50 = pool.tile([128, L], F32)
                s2r = pool.tile([128, 1], F32)
                nc.scalar.activation(out=scr50[:], in_=rows_t[:], func=ACTF.Square,
                                     bias=negmur[:], accum_out=s2r[:])

                osb = pool.tile([128, 6], F32)
                nc.scalar.activation(out=osb[:, 0:1], in_=sd[:], func=ACTF.Copy,
                                     scale=float(a) / 2500.0, bias=float(b))
                nc.scalar.activation(out=osb[:, 1:2], in_=s2[:], func=ACTF.Sqrt,
                                     scale=float(a) * float(a) / 2499.0)
                nc.vector.memset(osb[:, 2:3], float(b))
                q3 = pool.tile([128, 1], F32)
                nc.scalar.activation(out=q3[:], in_=maxd2[:], func=ACTF.Sqrt,
                                     scale=float(a) * float(a))
                nc.scalar.activation(out=osb[:, 3:4], in_=q3[:], func=ACTF.Copy,
                                     bias=float(b))
                nc.vector.memset(osb[:, 4:5], CONN_CONST)
                nc.scalar.activation(out=osb[:, 5:6], in_=s2r[:], func=ACTF.Sqrt,
                                     scale=float(a) * float(a) / 49.0)
                nc.sync.dma_start(out=out_t[g * 128:(g + 1) * 128], in_=osb[:])

                dsb = pool.tile([128, 2], F32)
                nc.vector.tensor_copy(out=dsb[:, 0:1], in_=c4[:])
                nc.vector.tensor_copy(out=dsb[:, 1:2], in_=cf[:])
                nc.sync.dma_start(out=dbg_t[g * 128:(g + 1) * 128], in_=dsb[:])

    nc.compile()
    return nc


def _const_inputs():
    return {
        "iotaneg_c": -np.arange(N, dtype=np.float32)[None, :],
        "iota24_c": np.arange(8 * WIN_ROUNDS, dtype=np.float32)[None, :],
        "rowb_c": (np.arange(128, dtype=np.float32) * N)[:, None],
    }


def _numpy_fallback(pc, consts):
    """Degenerate-parameter path (sigma==0 or a==0). Exact, CPU."""
    B = pc.shape[0]
    a, b = consts["a"], consts["b"]
    w = np.array([consts["w0"], consts["w1"]], np.float32)
    out = np.zeros((B, 6), np.float32)
    for i in range(B):
        s = pc[i] @ w
        idx = np.argsort(-s, kind="stable")[:L]
        Lp = pc[i, np.sort(idx)]
        d = np.sqrt(((Lp[:, None] - Lp[None, :]) ** 2).sum(-1))
        sc = d * a + b
        fl = np.sort(sc.ravel())
        med = fl[(L * L - 1) // 2]
        out[i] = [sc.mean(), sc.std(ddof=1), fl[0], fl[-1],
                  (sc < med).mean(), sc.sum(1).std(ddof=1)]
    return out


N_LAUNCH = 2  # sequential NEFF launches; 32 groups of gathers in one NEFF
              # overflows the qPoolDynamic semaphore budget and wedges the NC


def _get_runner(consts, nb):
    """Build (once) the Bacc program + a jitted 8-core sharded callable."""
    key = (consts["w0"], consts["w1"], consts["a"], consts["b"], nb)
    if key in _CACHE:
        return _CACHE[key]

    import jax
    from jax.sharding import Mesh, PartitionSpec, NamedSharding
    from jax.experimental.shard_map import shard_map
    import concourse.bass2jax as b2j
    import concourse.mybir as mybir

    nc = _build_program(consts, nb)
    b2j.install_neuronx_cc_hook()

    in_names, out_names, out_avals, zeros = [], [], [], []
    misc_inputs = {}
    for alloc in nc.m.functions[0].allocations:
        if not isinstance(alloc, mybir.MemoryLocationSet):
            continue
        name = alloc.memorylocations[0].name
        if alloc.kind == "ExternalInput":
            in_names.append(name)
            misc_inputs[name] = (tuple(alloc.tensor_shape),
                                 mybir.dt.np(alloc.dtype))
        elif alloc.kind == "ExternalOutput":
            out_names.append(name)
            shape = tuple(alloc.tensor_shape)
            dtype = mybir.dt.np(alloc.dtype)
            out_avals.append(jax.core.ShapedArray(shape, dtype))
            zeros.append(np.zeros(shape, dtype))
    n_params = len(in_names)
    all_names = in_names + out_names

    def _body(*args):
        return tuple(b2j._bass_exec_p.bind(
            *args, out_avals=tuple(out_avals), in_names=tuple(all_names),
            out_names=tuple(out_names), lowering_input_output_aliases=(),
            sim_require_finite=False, sim_require_nnan=False, nc=nc))

    devices = jax.devices()[:N_CORES]
    mesh = Mesh(np.asarray(devices), ("core",))
    spec = PartitionSpec("core")
    fn = jax.jit(shard_map(
        _body, mesh=mesh, in_specs=(spec,) * (n_params + len(out_names)),
        out_specs=(spec,) * len(out_names), check_rep=False))
    runner = dict(nc=nc, fn=fn, in_names=in_names, out_names=out_names,
                  zeros=zeros, mesh=mesh, spec=spec, misc_inputs=misc_inputs)
    _CACHE[key] = runner
    return runner


def run_shards(runner, pc_shards):
    """Execute one launch: pc_shards is a list of N_CORES per-core arrays."""
    import jax
    from jax.sharding import NamedSharding

    cin = _const_inputs()
    sharding = NamedSharding(runner["mesh"], runner["spec"])
    gather = []
    for name in runner["in_names"]:
        if name == "pc":
            gather.append(np.concatenate(pc_shards, axis=0))
        elif name not in cin:  # partition_id etc. — unused, any value works
            shape, dtype = runner["misc_inputs"][name]
            gather.append(np.zeros((shape[0] * N_CORES,) + tuple(shape[1:]),
                                   dtype))
        else:
            gather.append(np.concatenate([cin[name]] * N_CORES, axis=0))
    gather += [np.concatenate([z] * N_CORES, axis=0) for z in runner["zeros"]]
    args = [jax.device_put(a, sharding) for a in gather]
    outs = runner["fn"](*args)
    outs = [np.asarray(o) for o in outs]
    return dict(zip(runner["out_names"], outs))


def kernel(point_cloud, attn_w, attn_b, filtration_weights, distance_bias):
    pc = np.ascontiguousarray(np.asarray(point_cloud, dtype=np.float32))
    B = pc.shape[0]
    consts = _host_constants(attn_w, filtration_weights, distance_bias)
    if consts["sigma"] == 0.0 or consts["a"] == 0.0:
        return _numpy_fallback(pc, consts)

    nb_core = B // N_CORES           # rows per core overall
    nb = nb_core // N_LAUNCH         # rows per core per launch
    runner = _get_runner(consts, nb)

    outs, dbgs = [], []
    for h in range(N_LAUNCH):
        shards = [pc[c * nb_core + h * nb: c * nb_core + (h + 1) * nb]
                  for c in range(N_CORES)]
        res = run_shards(runner, shards)
        outs.append(res["out"].reshape(N_CORES, nb, 6))
        dbgs.append(res["dbg"].reshape(N_CORES, nb, 2))
    out = np.concatenate(outs, axis=1).reshape(B, 6)
    LAST["dbg"] = np.concatenate(dbgs, axis=1).reshape(B, 2)
    LAST["exec_time_ns"] = None
    return out



# revision 4
# speedup vs baseline: 1.7579x; 1.7579x over previous
"""TRN2 Bass kernel v3 for nn_DifferentiablePersistentHomology_90933047591278.

kernel(**inputs) takes the FULL inputs (point_cloud [32768,1024,2] f32 plus
the tiny learned params) and returns the FULL [32768, 6] f32 output,
computed on 8 NeuronCores (pure batch data-parallel, 4096 rows per core).

Per 128-row group (one row per SBUF partition), all in ONE NEFF launch:
  s' = -(w0*x + w1*y)  ->  exact rank-50 threshold T' via 4 ACT
  Sign-counting passes (Newton chain; counts are exact +/-1 sums) + a
  32-wide DVE max8/match_replace window  ->  mask = (s' <= T')  ->  DVE
  tensor_tensor_scan prefix-sum ranks  ->  ONE gpsimd local_scatter of
  interleaved fp16 (x,y) pairs (no DRAM gather)  ->  fp16 50x50 distance
  stats with fused accumulations.

Engine split respects walrus legality: Pool runs only plain tensor_scalar
forms + local_scatter (single gpsimd library); scan/stt/reduce on DVE;
Sign/Square/Sqrt/Relu/Copy on ACT (all within one act table set).

Outputs: [mean, std, min, max, conn, row_std]; min == distance_bias and
conn == 1248/2500 are exact constants (see baseline derivation).
"""
import os
import sys

if "/opt/trn_rl_repo" not in sys.path:
    sys.path.insert(0, "/opt/trn_rl_repo")

import numpy as np

N = 1024
L = 50
B_TOTAL = 32768
N_CORES = 8
NEG_BIG = -1e30
BIGF = 1e27
CONN_CONST = 1248.0 / 2500.0
BIAS3 = 10.0
WIN_ROUNDS = 4          # window of 32 ascending-s values above t3

LAST = {}
_CACHE = {}


def _host_constants(attn_w, filtration_weights, distance_bias, nsim=20000):
    w0 = float(np.asarray(attn_w)[0, 0])
    w1 = float(np.asarray(attn_w)[0, 1])
    sigma = float(np.hypot(w0, w1))
    a = abs(float(np.asarray(filtration_weights)[0, 0]))
    b = float(np.asarray(distance_bias)[0])
    if sigma == 0.0:
        return dict(w0=w0, w1=w1, sigma=sigma, a=a, b=b)
    t0 = sigma * 1.268
    sim = np.random.default_rng(1).standard_normal((nsim, N)).astype(np.float32) * sigma
    c1s = (sim >= t0).sum(axis=1)
    part = np.partition(sim, (N - L - 1, N - L), axis=1)
    tgt = 0.5 * (part[:, N - L] + part[:, N - L - 1])
    co = np.polyfit(c1s.astype(np.float64), tgt, 2)
    quad = (float(co[2]), float(co[1]), float(co[0]))
    g50 = sigma / (N * 0.10226)
    return dict(w0=w0, w1=w1, sigma=sigma, a=a, b=b, t0=t0, quad=quad, g50=g50)


def _build_program(consts, n_batches):
    import concourse.bass as bass
    import concourse.bacc as bacc
    import concourse.mybir as mybir
    from concourse.tile import TileContext

    F32 = mybir.dt.float32
    F16 = mybir.dt.float16
    I16 = mybir.dt.int16
    ALU = mybir.AluOpType
    ACTF = mybir.ActivationFunctionType

    assert n_batches % 128 == 0
    groups = n_batches // 128
    # negated-score domain: s' = -(w0 x + w1 y); top-50 of s == bottom-50 of s'
    w0n, w1n = -consts["w0"], -consts["w1"]
    a, b = consts["a"], consts["b"]
    t0n = -consts["t0"]
    A0, A1, A2 = consts["quad"]
    A0n, A1n, A2n = -A0, -A1, -A2
    g50n = -consts["g50"]
    W = 8 * WIN_ROUNDS

    nc = bacc.Bacc()
    pc = nc.dram_tensor("pc", [n_batches, N, 2], F32, kind="ExternalInput")
    iota32_c = nc.dram_tensor("iota32_c", [1, W], F32, kind="ExternalInput")
    out_t = nc.dram_tensor("out", [n_batches, 6], F32, kind="ExternalOutput")
    dbg_t = nc.dram_tensor("dbg", [n_batches, 2], F32, kind="ExternalOutput")

    with TileContext(nc) as tc:
        with tc.tile_pool(name="const", bufs=1) as cpool, \
             tc.tile_pool(name="work", bufs=2) as pool:
            iota32 = cpool.tile([128, W], F32)
            nc.sync.dma_start(out=iota32[:], in_=iota32_c.ap().broadcast_to([128, W]))
            t0b = cpool.tile([128, 1], F32)
            nc.vector.memset(t0b[:], float(t0n))

            for g in range(groups):
                xt = pool.tile([128, N, 2], F32)
                nc.sync.dma_start(out=xt[:], in_=pc[g * 128:(g + 1) * 128])

                # s' = (x * w0n) + (y * w1n)
                ty = pool.tile([128, N], F32, tag="big0")
                nc.gpsimd.tensor_scalar(out=ty[:], in0=xt[:, :, 1],
                                        scalar1=float(w1n), scalar2=None,
                                        op0=ALU.mult)
                s = pool.tile([128, N], F32)
                nc.vector.scalar_tensor_tensor(
                    out=s[:], in0=xt[:, :, 0], scalar=float(w0n), in1=ty[:],
                    op0=ALU.mult, op1=ALU.add)

                # Newton chain: counts via ACT Sign sums (exact integers)
                # count_le(s', t) = (N + sum sign(t - s')) / 2
                junk = pool.tile([128, N], F32, tag="big1")
                ss1 = pool.tile([128, 1], F32)
                nc.scalar.activation(out=junk[:], in_=s[:], func=ACTF.Sign,
                                     scale=-1.0, bias=t0b[:],
                                     accum_out=ss1[:])
                c1 = pool.tile([128, 1], F32)
                nc.gpsimd.tensor_scalar(out=c1[:], in0=ss1[:], scalar1=0.5,
                                        scalar2=float(N) / 2.0, op0=ALU.mult,
                                        op1=ALU.add)
                u1 = pool.tile([128, 1], F32)
                nc.gpsimd.tensor_scalar(out=u1[:], in0=c1[:], scalar1=float(A2n),
                                        scalar2=float(A1n), op0=ALU.mult,
                                        op1=ALU.add)
                t1 = pool.tile([128, 1], F32)
                nc.scalar.activation(out=t1[:], in_=c1[:], func=ACTF.Copy,
                                     scale=u1[:], bias=float(A0n))
                ss2 = pool.tile([128, 1], F32)
                nc.scalar.activation(out=junk[:], in_=s[:], func=ACTF.Sign,
                                     scale=-1.0, bias=t1[:], accum_out=ss2[:])
                # t2 = t1 + (c2 - 50)*g50n ; c2 = ss2/2 + 512
                v1 = pool.tile([128, 1], F32)
                nc.gpsimd.tensor_scalar(out=v1[:], in0=ss2[:],
                                        scalar1=float(g50n) / 2.0,
                                        scalar2=float((N / 2.0 - L) * g50n),
                                        op0=ALU.mult, op1=ALU.add)
                t2 = pool.tile([128, 1], F32)
                nc.vector.tensor_scalar(out=t2[:], in0=v1[:], scalar1=t1[:],
                                        scalar2=None, op0=ALU.add)
                ss3 = pool.tile([128, 1], F32)
                nc.scalar.activation(out=junk[:], in_=s[:], func=ACTF.Sign,
                                     scale=-1.0, bias=t2[:], accum_out=ss3[:])
                v2 = pool.tile([128, 1], F32)
                nc.gpsimd.tensor_scalar(out=v2[:], in0=ss3[:],
                                        scalar1=float(g50n) / 2.0,
                                        scalar2=float((N / 2.0 - L - BIAS3) * g50n),
                                        op0=ALU.mult, op1=ALU.add)
                t3 = pool.tile([128, 1], F32)
                nc.vector.tensor_scalar(out=t3[:], in0=v2[:], scalar1=t2[:],
                                        scalar2=None, op0=ALU.add)
                ss4 = pool.tile([128, 1], F32)
                nc.scalar.activation(out=junk[:], in_=s[:], func=ACTF.Sign,
                                     scale=-1.0, bias=t3[:], accum_out=ss4[:])
                # rk = c4 - 50 = ss4/2 + 512 - 50  (exact integer)
                rk = pool.tile([128, 1], F32)
                nc.gpsimd.tensor_scalar(out=rk[:], in0=ss4[:], scalar1=0.5,
                                        scalar2=float(N / 2.0 - L),
                                        op0=ALU.mult, op1=ALU.add)

                # window: z0 = s' - BIGF*relu(s' - t3)  (== s' for candidates,
                # huge negative for non-candidates; candidates stay EXACT)
                t3m = pool.tile([128, 1], F32)
                nc.gpsimd.tensor_scalar(out=t3m[:], in0=t3[:], scalar1=-1.0,
                                        scalar2=None, op0=ALU.mult)
                r2 = pool.tile([128, N], F32, tag="big0")
                nc.scalar.activation(out=r2[:], in_=s[:], func=ACTF.Relu,
                                     bias=t3m[:])
                z0 = pool.tile([128, N], F32, tag="big2")
                nc.vector.scalar_tensor_tensor(
                    out=z0[:], in0=r2[:], scalar=-BIGF, in1=s[:],
                    op0=ALU.mult, op1=ALU.add)
                w32 = pool.tile([128, W], F32)
                zc = z0
                for r in range(WIN_ROUNDS):
                    nc.vector.max(out=w32[:, 8 * r:8 * (r + 1)], in_=zc[:])
                    if r + 1 < WIN_ROUNDS:
                        zn = pool.tile([128, N], F32, tag=f"zr{r % 2}")
                        nc.vector.match_replace(
                            out=zn[:], in_to_replace=w32[:, 8 * r:8 * (r + 1)],
                            in_values=zc[:], imm_value=NEG_BIG)
                        zc = zn

                # T' = w32[rk]
                eqm = pool.tile([128, W], F32)
                nc.vector.scalar_tensor_tensor(
                    out=eqm[:], in0=iota32[:], scalar=rk[:], in1=w32[:],
                    op0=ALU.is_equal, op1=ALU.mult)
                tn = pool.tile([128, 1], F32)
                nc.vector.tensor_reduce(out=tn[:], in_=eqm[:],
                                        axis=mybir.AxisListType.X, op=ALU.add)

                # exact top-50 mask (s' <= T')
                maskf = pool.tile([128, N], F32)
                nc.gpsimd.tensor_scalar(out=maskf[:], in0=s[:], scalar1=tn[:],
                                        scalar2=None, op0=ALU.is_le)

                # ranks: inclusive prefix sum; m1 = scan*mask;
                # pair indices: even slots 2*(m1-1), odd slots 2*(m1-1)+1,
                # negatives (from m1==0) are ignored by local_scatter
                scan = pool.tile([128, N], F32, tag="big1")
                nc.vector.tensor_tensor_scan(
                    out=scan[:], data0=maskf[:], data1=maskf[:], initial=0.0,
                    op0=ALU.add, op1=ALU.bypass)
                m1 = pool.tile([128, N], F32, tag="big0")
                nc.vector.scalar_tensor_tensor(
                    out=m1[:], in0=scan[:], scalar=0.0, in1=maskf[:],
                    op0=ALU.add, op1=ALU.mult)
                idxs2 = pool.tile([128, N, 2], I16)
                nc.gpsimd.tensor_scalar(out=idxs2[:, :, 0], in0=m1[:],
                                        scalar1=2.0, scalar2=-2.0,
                                        op0=ALU.mult, op1=ALU.add)
                nc.gpsimd.tensor_scalar(out=idxs2[:, :, 1], in0=m1[:],
                                        scalar1=2.0, scalar2=-1.0,
                                        op0=ALU.mult, op1=ALU.add)
                # m1==0 (unselected) -> -2/-1, ignored; m1=r in 1..50 ->
                # x half at 2(r-1), y half at 2(r-1)+1

                # fp16 interleaved coords + ONE local scatter of pairs
                xy16 = pool.tile([128, N, 2], F16)
                nc.scalar.activation(
                    out=xy16[:].rearrange("p n c -> p (n c)"),
                    in_=xt[:].rearrange("p n c -> p (n c)"), func=ACTF.Copy)
                xyc = pool.tile([128, 128], F16)
                nc.gpsimd.local_scatter(
                    out_ap=xyc[:],
                    data_ap=xy16[:].rearrange("p n c -> p (n c)"),
                    idxs_ap=idxs2[:].rearrange("p n c -> p (n c)"),
                    channels=128, num_elems=128, num_idxs=2 * N)

                # 50x50 pairwise distances in fp16 (one fused subtract)
                pcv = xyc[:].rearrange("p (k c) -> p k c", c=2)
                pi = pcv[:, 0:L, :].unsqueeze(2).broadcast_to([128, L, L, 2])
                pj = pcv[:, 0:L, :].unsqueeze(1).broadcast_to([128, L, L, 2])
                dxy = pool.tile([128, L, L, 2], F16, tag="pair0")
                nc.vector.tensor_tensor(out=dxy[:], in0=pi, in1=pj,
                                        op=ALU.subtract)
                sq = pool.tile([128, L, L, 2], F16, tag="pair1")
                nc.scalar.activation(
                    out=sq[:].rearrange("p i j c -> p (i j c)"),
                    in_=dxy[:].rearrange("p i j c -> p (i j c)"),
                    func=ACTF.Square)
                d2t = pool.tile([128, L, L], F16, tag="pair2")
                nc.vector.tensor_tensor(out=d2t[:], in0=sq[:, :, :, 0],
                                        in1=sq[:, :, :, 1], op=ALU.add)
                sd2 = pool.tile([128, 1], F32)
                nc.vector.tensor_reduce(out=sd2[:], in_=d2t[:].rearrange(
                    "p a b -> p (a b)"), axis=mybir.AxisListType.X, op=ALU.add)
                dist = pool.tile([128, L, L], F16, tag="pair0")
                sd = pool.tile([128, 1], F32)
                nc.scalar.activation(out=dist[:], in_=d2t[:], func=ACTF.Sqrt,
                                     accum_out=sd[:])
                maxd2 = pool.tile([128, 1], F32)
                nc.vector.tensor_reduce(out=maxd2[:], in_=d2t[:].rearrange(
                    "p a b -> p (a b)"), axis=mybir.AxisListType.X, op=ALU.max)
                rows_t = pool.tile([128, L], F32)
                nc.vector.tensor_reduce(out=rows_t[:], in_=dist[:],
                                        axis=mybir.AxisListType.X, op=ALU.add)

                # output stats
                osb = pool.tile([128, 6], F32)
                nc.scalar.activation(out=osb[:, 0:1], in_=sd[:], func=ACTF.Copy,
                                     scale=float(a) / 2500.0, bias=float(b))
                q1 = pool.tile([128, 1], F32)
                nc.scalar.activation(out=q1[:], in_=sd[:], func=ACTF.Square,
                                     scale=1.0 / 50.0)
                s2 = pool.tile([128, 1], F32)
                nc.vector.tensor_tensor(out=s2[:], in0=sd2[:], in1=q1[:],
                                        op=ALU.subtract)
                nc.scalar.activation(out=osb[:, 1:2], in_=s2[:], func=ACTF.Sqrt,
                                     scale=float(a) * float(a) / 2499.0)
                nc.vector.memset(osb[:, 2:3], float(b))
                q3 = pool.tile([128, 1], F32)
                nc.scalar.activation(out=q3[:], in_=maxd2[:], func=ACTF.Sqrt,
                                     scale=float(a) * float(a))
                nc.scalar.activation(out=osb[:, 3:4], in_=q3[:], func=ACTF.Copy,
                                     bias=float(b))
                nc.vector.memset(osb[:, 4:5], CONN_CONST)
                negmur = pool.tile([128, 1], F32)
                nc.scalar.activation(out=negmur[:], in_=sd[:], func=ACTF.Copy,
                                     scale=-1.0 / float(L))
                scr50 = pool.tile([128, L], F32)
                s2r = pool.tile([128, 1], F32)
                nc.scalar.activation(out=scr50[:], in_=rows_t[:], func=ACTF.Square,
                                     bias=negmur[:], accum_out=s2r[:])
                nc.scalar.activation(out=osb[:, 5:6], in_=s2r[:], func=ACTF.Sqrt,
                                     scale=float(a) * float(a) / 49.0)
                nc.sync.dma_start(out=out_t[g * 128:(g + 1) * 128], in_=osb[:])

                # debug: c4 (=rk+50) and cf (=scan[-1], the exact-mask count)
                dsb = pool.tile([128, 2], F32)
                nc.vector.tensor_scalar(out=dsb[:, 0:1], in0=rk[:],
                                        scalar1=float(L), scalar2=None,
                                        op0=ALU.add)
                nc.vector.tensor_copy(out=dsb[:, 1:2], in_=scan[:, N - 1:N])
                nc.sync.dma_start(out=dbg_t[g * 128:(g + 1) * 128], in_=dsb[:])

    nc.compile()
    return nc


def _const_inputs():
    return {
        "iota32_c": np.arange(8 * WIN_ROUNDS, dtype=np.float32)[None, :],
    }


def _numpy_fallback(pc, consts):
    """Degenerate-parameter path (sigma==0 or a==0). Exact, CPU."""
    B = pc.shape[0]
    a, b = consts["a"], consts["b"]
    w = np.array([consts["w0"], consts["w1"]], np.float32)
    out = np.zeros((B, 6), np.float32)
    for i in range(B):
        s = pc[i] @ w
        idx = np.argsort(-s, kind="stable")[:L]
        Lp = pc[i, np.sort(idx)]
        d = np.sqrt(((Lp[:, None] - Lp[None, :]) ** 2).sum(-1))
        sc = d * a + b
        fl = np.sort(sc.ravel())
        med = fl[(L * L - 1) // 2]
        out[i] = [sc.mean(), sc.std(ddof=1), fl[0], fl[-1],
                  (sc < med).mean(), sc.sum(1).std(ddof=1)]
    return out


N_LAUNCH = 1


def _get_runner(consts, nb):
    """Build (once) the Bacc program + a jitted 8-core sharded callable."""
    key = (consts["w0"], consts["w1"], consts["a"], consts["b"], nb)
    if key in _CACHE:
        return _CACHE[key]

    import jax
    from jax.sharding import Mesh, PartitionSpec
    from jax.experimental.shard_map import shard_map
    import concourse.bass2jax as b2j
    import concourse.mybir as mybir

    nc = _build_program(consts, nb)
    b2j.install_neuronx_cc_hook()

    in_names, out_names, out_avals, zeros = [], [], [], []
    misc_inputs = {}
    for alloc in nc.m.functions[0].allocations:
        if not isinstance(alloc, mybir.MemoryLocationSet):
            continue
        name = alloc.memorylocations[0].name
        if alloc.kind == "ExternalInput":
            in_names.append(name)
            misc_inputs[name] = (tuple(alloc.tensor_shape),
                                 mybir.dt.np(alloc.dtype))
        elif alloc.kind == "ExternalOutput":
            out_names.append(name)
            shape = tuple(alloc.tensor_shape)
            dtype = mybir.dt.np(alloc.dtype)
            out_avals.append(jax.core.ShapedArray(shape, dtype))
            zeros.append(np.zeros(shape, dtype))
    n_params = len(in_names)
    all_names = in_names + out_names

    def _body(*args):
        return tuple(b2j._bass_exec_p.bind(
            *args, out_avals=tuple(out_avals), in_names=tuple(all_names),
            out_names=tuple(out_names), lowering_input_output_aliases=(),
            sim_require_finite=False, sim_require_nnan=False, nc=nc))

    devices = jax.devices()[:N_CORES]
    mesh = Mesh(np.asarray(devices), ("core",))
    spec = PartitionSpec("core")
    fn = jax.jit(shard_map(
        _body, mesh=mesh, in_specs=(spec,) * (n_params + len(out_names)),
        out_specs=(spec,) * len(out_names), check_rep=False))
    runner = dict(nc=nc, fn=fn, in_names=in_names, out_names=out_names,
                  zeros=zeros, mesh=mesh, spec=spec, misc_inputs=misc_inputs)
    _CACHE[key] = runner
    return runner


def run_shards(runner, pc_shards):
    """Execute one launch: pc_shards is a list of N_CORES per-core arrays."""
    import jax
    from jax.sharding import NamedSharding

    cin = _const_inputs()
    sharding = NamedSharding(runner["mesh"], runner["spec"])
    gather = []
    for name in runner["in_names"]:
        if name == "pc":
            gather.append(np.concatenate(pc_shards, axis=0))
        elif name not in cin:  # partition_id etc. — unused, any value works
            shape, dtype = runner["misc_inputs"][name]
            gather.append(np.zeros((shape[0] * N_CORES,) + tuple(shape[1:]),
                                   dtype))
        else:
            gather.append(np.concatenate([cin[name]] * N_CORES, axis=0))
    gather += [np.concatenate([z] * N_CORES, axis=0) for z in runner["zeros"]]
    args = [jax.device_put(a, sharding) for a in gather]
    outs = runner["fn"](*args)
    outs = [np.asarray(o) for o in outs]
    return dict(zip(runner["out_names"], outs))


def kernel(point_cloud, attn_w, attn_b, filtration_weights, distance_bias):
    pc = np.ascontiguousarray(np.asarray(point_cloud, dtype=np.float32))
    B = pc.shape[0]
    consts = _host_constants(attn_w, filtration_weights, distance_bias)
    if consts["sigma"] == 0.0 or consts["a"] == 0.0:
        return _numpy_fallback(pc, consts)

    nb_core = B // N_CORES           # rows per core overall
    nb = nb_core // N_LAUNCH         # rows per core per launch
    runner = _get_runner(consts, nb)

    outs, dbgs = [], []
    for h in range(N_LAUNCH):
        shards = [pc[c * nb_core + h * nb: c * nb_core + (h + 1) * nb]
                  for c in range(N_CORES)]
        res = run_shards(runner, shards)
        outs.append(res["out"].reshape(N_CORES, nb, 6))
        dbgs.append(res["dbg"].reshape(N_CORES, nb, 2))
    out = np.concatenate(outs, axis=1).reshape(B, 6)
    LAST["dbg"] = np.concatenate(dbgs, axis=1).reshape(B, 2)
    LAST["exec_time_ns"] = None
    return out


# ---------------------------------------------------------------------------
# dev: interpreter validation on a small slice (no HW, no neuronxcc)
if __name__ == "__main__":
    import jax
    jax.config.update("jax_default_device", jax.devices("cpu")[0])
    sys.path.insert(0, "/root/problem")
    import reference as ref_mod
    from concourse.bass_interp import MultiCoreSim

    inputs = {k: np.asarray(v) for k, v in ref_mod.setup_inputs().items()}
    consts = _host_constants(inputs["attn_w"], inputs["filtration_weights"],
                             inputs["distance_bias"])
    NB = 256  # 2 groups
    pc = np.ascontiguousarray(inputs["point_cloud"][:NB])

    nc = _build_program(consts, NB)
    nc.insert_bir_kernel_barrier_sem_inc()
    sim = MultiCoreSim(nc, 1, require_finite=False, require_nnan=False)
    core = sim.cores[0]
    core.tensor("pc")[:] = pc
    for k, v in _const_inputs().items():
        core.tensor(k)[:] = v
    sim.simulate()
    actual = np.array(core.tensor("out"))
    dbg = np.array(core.tensor("dbg"))
    print("c4 range", dbg[:, 0].min(), dbg[:, 0].max(),
          "cf!=50:", (dbg[:, 1] != 50).sum())
    print("modeled time (ns):", sim.global_time, " per group:",
          sim.global_time / (NB // 128))

    sub = {k: (v[:NB] if k == "point_cloud" else v) for k, v in inputs.items()}
    expected = np.asarray(ref_mod.reference(**{k: np.asarray(v) for k, v in sub.items()}))
    rel = np.abs(actual - expected) / np.maximum(np.abs(expected), 1e-6)
    print("per-col max rel:", np.array2string(rel.max(axis=0), precision=2))
    i = np.unravel_index(rel.argmax(), rel.shape)
    print(f"worst: row {i[0]} col {i[1]} act={actual[i]} exp={expected[i]}")
    print(f"Relative error: {rel.max():.6e}")
